# revision 1
# baseline (speedup 1.0000x reference)
"""Trainium2 Bass kernel: soft-top-k-masked pseudo-diagonal fully connected layer.

Computes, for x [16, 1024, 768], V [2304, 768], alpha [2304]:
    m  = dykstra_capped_simplex(alpha / 0.01, k=231, 50 iters)        # [2304]
    W[o, j] = m[(o - j) % 2304] * V[(o - j) % 2304, j]                # [2304, 768]
    out = x @ W.T                                                     # [16, 1024, 2304]

Key identities used:
  * Dykstra on the capped simplex reduces to a scalar recursion on w:
        w_1     = z + (k - sum(z)) / n
        w_{i+1} = w_i + (k - sum(clip(w_i, 0, 1))) / n     (49 times)
        m       = clip(w_50, 0, 1)
    (p is always a constant vector and y+q == w, so only w and the scalar
    sum survive.)  clip+sum fuse into one DVE op (scalar_tensor_tensor with
    accum_out); the cross-partition sum broadcast is a ones-matmul on the PE.
  * The scatter .at[rows, cols].add is a bijection per column, so
        W.T[j, o] = (m * V).T[j, (o - j) mod 2304]
    i.e. row j of W.T is row j of V.T cyclically shifted right by j, scaled
    by a shifted copy of m.  V.T is stored wrap-extended in DRAM as
    ext[768, 2432] with ext[j, 128 + c] = V.T[j, c mod 2304]; a W.T j-tile
    is then two DMAs with partition stride 2431 (skewed access pattern).
    This raw roll is *independent of m*, so it runs concurrently with the
    Dykstra iteration; m is applied afterwards as
        W.T_tile[b] *= m_skew[b],   m_skew[b][dj, o] = m[(o-128b-dj) % 2304]
    where m_skew is a skewed broadcast of m loaded from a 130x-replicated
    m_rep buffer with partition stride 2303 (== -1 mod 2304).

Sharding: data-parallel over the 16384 tokens -> 2048 tokens per core on 8
cores; V/alpha replicated (per the sharding hint). The x/V/W datapath is
float32r (fp22 on the PE; 1 cycle/row for moving dims >= 256, 1.5
cycles/row transposes); the Dykstra recursion stays exact float32.
"""

import numpy as np

from concourse import bass, bacc, mybir, tile
from concourse import bass_utils
from concourse.ap import AP

F32 = mybir.dt.float32
F32R = mybir.dt.float32r

N_CORES = 8
T_FULL = 16 * 1024          # total tokens
T = T_FULL // N_CORES       # tokens per core = 2048
D = 768                     # in features (contraction)
O = 2304                    # out features
P = 2304                    # total perm (mask length)
PAD = 128                   # ext left wrap pad (covers the intra-tile skew)
EXTW = P + PAD              # 2432
KTOP = 231                  # top-k target
NUM_ITER = 50
INV_LR = 100.0              # 1 / 0.01
K_OVER_N = np.float64(KTOP) / np.float64(P)  # added as fp32 imm by bass

NT = T // 128               # 16 token tiles per core
NJ = D // 128               # 6 contraction tiles
NP = P // 128               # 18 mask/V row tiles
# o-chunks for the main matmul (one PSUM bank each, >=256 for full-rate f32r)
O_CHUNKS = [(0, 512), (512, 1024), (1024, 1536), (1536, 2048), (2048, 2304)]


def build_program():
    nc = bacc.Bacc("TRN2", target_bir_lowering=False, debug=False,
                   num_devices=N_CORES)

    # x/V/ident carry f32 bits; typing them f32r makes the whole transpose +
    # matmul datapath f32r-native (single-pass on the PE).
    x_d = nc.dram_tensor("x", [T, D], F32R, kind="ExternalInput")
    v_d = nc.dram_tensor("v", [P, D], F32R, kind="ExternalInput")
    alpha_d = nc.dram_tensor("alpha", [P], F32, kind="ExternalInput")
    ident_d = nc.dram_tensor("ident", [128, 128], F32R, kind="ExternalInput")
    out_d = nc.dram_tensor("out", [T, O], F32, kind="ExternalOutput")

    ext_d = nc.dram_tensor("ext", [D, EXTW], F32R, kind="Internal")
    mtmp_d = nc.dram_tensor("m_tmp", [P], F32, kind="Internal")
    mrep_d = nc.dram_tensor("m_rep", [130 * P], F32, kind="Internal")

    x_r = x_d.ap().rearrange("(n p) j -> n p j", p=128)      # [16, 128, 768]
    v_r = v_d.ap().rearrange("(n p) j -> n p j", p=128)      # [18, 128, 768]
    out_r = out_d.ap().rearrange("(n p) o -> n p o", p=128)  # [16, 128, 2304]
    alpha_nat = alpha_d.ap().rearrange("(f p) -> f p", p=128)  # [18, 128] rows

    with tile.TileContext(nc) as tc:
        with (
            tc.tile_pool(name="const", bufs=1) as constp,
            tc.tile_pool(name="small", bufs=1) as small,
            tc.tile_pool(name="xstage", bufs=4) as xstage,
            tc.tile_pool(name="vstage", bufs=3) as vstage,
            tc.tile_pool(name="xt", bufs=NJ) as xtp,
            tc.tile_pool(name="vtp", bufs=NJ) as vtp,
            tc.tile_pool(name="wtp", bufs=NJ) as wtp,
            tc.tile_pool(name="orow", bufs=2) as orow,
            tc.tile_pool(name="ps8", bufs=7, space="PSUM") as ps8,
            tc.tile_pool(name="dk", bufs=1, space="PSUM") as dkp,
        ):
            # ---- constants ----
            ident = constp.tile([128, 128], F32R)
            nc.sync.dma_start(ident[:], ident_d.ap())
            ones_inv_n = constp.tile([128, 128], F32)
            nc.vector.memset(ones_inv_n[:], 1.0 / float(P))
            zeros_t = constp.tile([128, NP], F32)
            nc.vector.memset(zeros_t[:], 0.0)

            # ---- Dykstra (critical path; gpsimd DMA ring keeps it isolated)
            alpha_nat_t = small.tile([18, 128], F32, tag="alnat")
            nc.gpsimd.dma_start(alpha_nat_t[:], alpha_nat)
            al_ps = dkp.tile([128, 18], F32, tag="dk")
            nc.tensor.transpose(al_ps[:], alpha_nat_t[:],
                                ident[0:18, 0:18].bitcast(F32))
            w = small.tile([128, NP], F32, tag="w")
            c = small.tile([128, NP], F32, tag="c")
            red = small.tile([128, 1], F32, tag="red")
            m_t = small.tile([128, NP], F32, tag="m")

            # w = 100*alpha ; w += (k - sum(w))/n
            nc.vector.tensor_scalar_mul(w[:], al_ps[:], INV_LR)
            nc.vector.reduce_sum(red[:], w[:], axis=mybir.AxisListType.X)
            s_ps = dkp.tile([128, 1], F32, tag="dk")
            nc.tensor.matmul(s_ps[:], ones_inv_n[:], red[:], start=True, stop=True)
            nc.vector.tensor_scalar(w[:], w[:], s_ps[:], K_OVER_N,
                                    op0=mybir.AluOpType.subtract,
                                    op1=mybir.AluOpType.add)
            for _ in range(NUM_ITER - 1):
                # c = clip(w, 0, 1); red = sum(c)   (single fused DVE op)
                nc.vector.scalar_tensor_tensor(c[:], w[:], 1.0, zeros_t[:],
                                               op0=mybir.AluOpType.min,
                                               op1=mybir.AluOpType.max,
                                               accum_out=red[:])
                s_ps = dkp.tile([128, 1], F32, tag="dk")
                nc.tensor.matmul(s_ps[:], ones_inv_n[:], red[:],
                                 start=True, stop=True)
                nc.vector.tensor_scalar(w[:], w[:], s_ps[:], K_OVER_N,
                                        op0=mybir.AluOpType.subtract,
                                        op1=mybir.AluOpType.add)
            nc.vector.tensor_scalar(m_t[:], w[:], 1.0, 0.0,
                                    op0=mybir.AluOpType.min,
                                    op1=mybir.AluOpType.max)

            # ---- m -> m_ext DRAM (wrap-extended, via PE transpose) ----
            mt_ps = dkp.tile([18, 128], F32, tag="dk")
            nc.tensor.transpose(mt_ps[:], m_t[:], ident[:].bitcast(F32))
            mt_sb = small.tile([18, 128], F32, tag="mtsb")
            nc.vector.tensor_copy(mt_sb[:], mt_ps[:])
            # m_tmp = m_vec (9KB contiguous), then one DRAM->DRAM DMA tiles
            # it 130x into m_rep (skew reads use positive partition stride
            # 2303 == -1 mod 2304)
            mw0 = nc.gpsimd.dma_start(
                mtmp_d.ap().rearrange("(f p) -> f p", p=128), mt_sb[:])
            mw1 = nc.gpsimd.dma_start(
                AP(mrep_d, 0, [[P, 130], [1, P]]),
                AP(mtmp_d, 0, [[0, 130], [1, P]]))
            tile.add_dep_helper(mw1.ins, mw0.ins, reason="m_tmp RAW")

            # ---- V load + transpose:  vt[b][j_local, p] = V[p, j0+j_local] ----
            vt = [vtp.tile([128, P], F32R, tag="vtp", name=f"vt{b}")
                  for b in range(NJ)]
            wt = [wtp.tile([128, P], F32R, tag="wtp", name=f"wt{b}")
                  for b in range(NJ)]
            cp_flip = 0
            for i in range(NP):
                v_t = vstage.tile([128, D], F32R, tag="vstage")
                nc.sync.dma_start(v_t[:], v_r[i])
                for b in range(NJ):
                    ps = ps8.tile([128, 128], F32R, tag="ps8")
                    nc.tensor.transpose(ps[:], v_t[:, 128 * b:128 * (b + 1)],
                                        ident[:])
                    dst = vt[b][:, 128 * i:128 * (i + 1)]
                    if cp_flip % 2 == 0:
                        nc.scalar.copy(dst, ps[:])
                    else:
                        nc.vector.tensor_copy(dst, ps[:])
                    cp_flip += 1

            # ---- x load + transpose:  xt[b][j_local, t] = x[t, j0+j_local] ----
            xt = [xtp.tile([128, T], F32R, tag="xt", name=f"xt{b}")
                  for b in range(NJ)]

            def x_tile_transpose(tt, flip):
                x_t = xstage.tile([128, D], F32R, tag="xstage", name=f"xs{tt}")
                nc.scalar.dma_start(x_t[:], x_r[tt])
                for b in range(NJ):
                    ps = ps8.tile([128, 128], F32R, tag="ps8", name=f"xps{tt}_{b}")
                    nc.tensor.transpose(ps[:], x_t[:, 128 * b:128 * (b + 1)],
                                        ident[:])
                    dst = xt[b][:, 128 * tt:128 * (tt + 1)]
                    if flip % 2 == 0:
                        nc.scalar.copy(dst, ps[:])
                    else:
                        nc.vector.tensor_copy(dst, ps[:])
                    flip += 1
                return flip

            for tt in range(NT):
                cp_flip = x_tile_transpose(tt, cp_flip)

            # ---- raw rolled weights: ext roundtrip (independent of m) ----
            ext_writes = []
            for b in range(NJ):
                j0 = 128 * b
                wmain = nc.sync.dma_start(ext_d.ap()[j0:j0 + 128, PAD:EXTW],
                                          vt[b][:])
                wwrap = nc.sync.dma_start(ext_d.ap()[j0:j0 + 128, 0:PAD],
                                          vt[b][:, P - PAD:P])
                ext_writes.append((wmain, wwrap))
            for b in range(NJ):
                j0 = 128 * b
                # piece A: wt[b][dj, o] for o in [j0, 2304):
                #   ext[j0+dj, PAD + (o - j0) - dj]
                skA = AP(ext_d, j0 * EXTW + PAD, [[EXTW - 1, 128], [1, P - j0]])
                rdA = nc.sync.dma_start(wt[b][:, j0:P], skA)
                tile.add_dep_helper(rdA.ins, ext_writes[b][0].ins, reason="extA")
                tile.add_dep_helper(rdA.ins, ext_writes[b][1].ins, reason="extAw")
                if j0 > 0:
                    # piece B: o in [0, j0): ext[j0+dj, PAD + (o + 2304 - j0) - dj]
                    skB = AP(ext_d, j0 * EXTW + PAD + (P - j0),
                             [[EXTW - 1, 128], [1, j0]])
                    rdB = nc.sync.dma_start(wt[b][:, 0:j0], skB)
                    tile.add_dep_helper(rdB.ins, ext_writes[b][0].ins, reason="extB")
                    tile.add_dep_helper(rdB.ins, ext_writes[b][1].ins, reason="extBw")

            # ---- skewed m broadcast + scale (after Dykstra) ----
            # m_skew[b][dj, o] = m_rep[dj*2303 + o + 2304 - j0]
            #                  = m_vec[(o - dj - j0) mod 2304]
            for b in range(NJ):
                j0 = 128 * b
                msk = vtp.tile([128, P], F32, tag="vtp", name=f"msk{b}")
                mr = nc.gpsimd.dma_start(
                    msk[:], AP(mrep_d, P - j0, [[P - 1, 128], [1, P]]))
                tile.add_dep_helper(mr.ins, mw1.ins, reason="m_rep RAW")
                nc.vector.tensor_tensor(wt[b][:], wt[b][:], msk[:],
                                        op=mybir.AluOpType.mult)

            # ---- main matmul: out[t, o] = sum_j x[t, j] * W.T[j, o] ----
            for tt in range(NT):
                row = orow.tile([128, O], F32, tag="orow")
                for ci, (o0, o1) in enumerate(O_CHUNKS):
                    ps = ps8.tile([128, 512], F32, tag="ps8")
                    cw = o1 - o0
                    for b in range(NJ):
                        nc.tensor.matmul(
                            ps[:, 0:cw],
                            xt[b][:, 128 * tt:128 * (tt + 1)],
                            wt[b][:, o0:o1],
                            start=(b == 0), stop=(b == NJ - 1),
                        )
                    if ci % 2 == 0:
                        nc.vector.tensor_copy(row[:, o0:o1], ps[:, 0:cw])
                    else:
                        nc.scalar.copy(row[:, o0:o1], ps[:, 0:cw])
                nc.scalar.dma_start(out_r[tt], row[:])

    nc.compile()
    return nc


_CACHE = {}


def _get_program():
    if "nc" not in _CACHE:
        _CACHE["nc"] = build_program()
    return _CACHE["nc"]


def kernel(x, V, alpha):
    nc = _get_program()
    xf = np.ascontiguousarray(x.reshape(T_FULL, D).astype(np.float32, copy=False))
    v = np.ascontiguousarray(V.astype(np.float32, copy=False))
    a = np.ascontiguousarray(alpha.astype(np.float32, copy=False))
    ident = np.eye(128, dtype=np.float32)
    in_maps = [
        {"x": xf[T * c:T * (c + 1)], "v": v, "alpha": a, "ident": ident}
        for c in range(N_CORES)
    ]
    res = bass_utils.run_bass_kernel_spmd(nc, in_maps, core_ids=list(range(N_CORES)))
    out = np.concatenate([res.results[c]["out"] for c in range(N_CORES)], axis=0)
    return out.reshape(16, 1024, O)



# revision 3
# speedup vs baseline: 1.2222x; 1.2222x over previous
"""Trainium2 Bass kernel: soft-top-k-masked pseudo-diagonal fully connected layer.

Computes, for x [16, 1024, 768], V [2304, 768], alpha [2304]:
    m  = dykstra_capped_simplex(alpha / 0.01, k=231, 50 iters)        # [2304]
    W[o, j] = m[(o - j) % 2304] * V[(o - j) % 2304, j]                # [2304, 768]
    out = x @ W.T                                                     # [16, 1024, 2304]

Key identities / structure:
  * Dykstra on the capped simplex reduces to a scalar recursion on w:
        w_1     = z + (k - sum(z)) / n
        w_{i+1} = w_i + (k - sum(clip(w_i, 0, 1))) / n     (49 times)
        m       = clip(w_50, 0, 1)
    Implemented on a [32, 72] layout: DVE clip+accum -> 32 partials,
    [32,32] ones-matmul on the PE sums+broadcasts them, DVE applies the
    per-partition scalar update.  The whole 50-iteration chain is the
    serial prefix of the kernel (the GEMM needs masked weights).
  * The scatter .at[rows, cols].add is a bijection per column, so
        W.T[j, o] = m[(o - j) % 2304] * V.T[j, (o - j) % 2304]
    i.e. row j of W.T is row j of V.T cyclically shifted right by j, scaled
    by a skewed broadcast of m.  The HOST pre-builds ext[768, 2432] fp16 with
    ext[j, 128 + c] = V.T[j, c mod 2304] (pure layout marshaling, no math);
    a raw W.T j-tile is then two DMAs with partition stride 2431 (skewed
    access pattern) straight from the input -- no on-device transposes and
    no DRAM round trip.  m is applied afterwards as
        wt[b] *= msk[b],   msk[b][dj, o] = m[(o - 128b - dj) % 2304]
    where msk is a skewed broadcast of m read from a 130x-replicated
    m_rep buffer with partition stride 2303 (== -1 mod 2304).
  * x is host-pre-transposed to xt [768, 2048] fp16 per core, so the
    16 x 5 x 6 fp16 matmul grid (PSUM f32 accumulate over the 6 j-blocks)
    starts as soon as the mask lands.  Output is stored fp16 and upcast on
    the host; fp16 end-to-end keeps rel err ~2e-3 << 2e-2 tolerance.

Sharding: data-parallel over the 16384 tokens -> 2048 tokens per core on 8
cores; V/alpha replicated (per the sharding hint). No collectives.
"""

import numpy as np

from concourse import bass, bacc, mybir, tile
from concourse import bass_utils
from concourse.ap import AP

F32 = mybir.dt.float32
F16 = mybir.dt.float16

N_CORES = 8
T_FULL = 16 * 1024          # total tokens
T = T_FULL // N_CORES       # tokens per core = 2048
D = 768                     # in features (contraction)
O = 2304                    # out features
P = 2304                    # total perm (mask length)
PAD = 128                   # ext left wrap pad (covers the intra-tile skew)
EXTW = P + PAD              # 2432
KTOP = 231                  # top-k target
NUM_ITER = 50
INV_LR = 100.0              # 1 / 0.01
K_OVER_N = np.float64(KTOP) / np.float64(P)

NT = T // 128               # 16 token tiles per core
NJ = D // 128               # 6 contraction tiles
# Dykstra layout: z as [DYK_P, DYK_F], flat index = q * DYK_F + r
DYK_P = 32
DYK_F = P // DYK_P          # 72
# o-chunks for the main matmul (one PSUM bank each)
O_CHUNKS = [(0, 512), (512, 1024), (1024, 1536), (1536, 2048), (2048, 2304)]


def build_program():
    nc = bacc.Bacc("TRN2", target_bir_lowering=False, debug=False,
                   num_devices=N_CORES)

    xt_d = nc.dram_tensor("xt", [D, T], F16, kind="ExternalInput")
    ext_d = nc.dram_tensor("ext", [D, EXTW], F16, kind="ExternalInput")
    al_d = nc.dram_tensor("al", [DYK_P, DYK_F], F32, kind="ExternalInput")
    out_d = nc.dram_tensor("out", [T, O], F16, kind="ExternalOutput")
    mrep_d = nc.dram_tensor("m_rep", [130 * P], F16, kind="Internal")

    out_r = out_d.ap().rearrange("(n p) o -> n p o", p=128)  # [16, 128, 2304]

    with tile.TileContext(nc) as tc:
        with (
            tc.tile_pool(name="small", bufs=1) as small,
            tc.tile_pool(name="xtp", bufs=NJ) as xtp,
            tc.tile_pool(name="wtp", bufs=NJ) as wtp,
            tc.tile_pool(name="mskp", bufs=NJ) as mskp,
            tc.tile_pool(name="orow", bufs=2) as orow,
            tc.tile_pool(name="ps8", bufs=7, space="PSUM") as ps8,
            tc.tile_pool(name="dk", bufs=1, space="PSUM") as dkp,
        ):
            # ---- x / raw-W.T loads (independent of the mask; start at t=0)
            xt = [xtp.tile([128, T], F16, tag="xt", name=f"xt{b}")
                  for b in range(NJ)]
            for b in range(NJ):
                nc.sync.dma_start(xt[b][:], xt_d.ap()[128 * b:128 * (b + 1), :])

            wt = [wtp.tile([128, P], F16, tag="wtp", name=f"wt{b}")
                  for b in range(NJ)]
            for b in range(NJ):
                j0 = 128 * b
                # piece A: wt[b][dj, o] for o in [j0, 2304):
                #   ext[j0+dj, PAD + (o - j0) - dj]
                skA = AP(ext_d, j0 * EXTW + PAD, [[EXTW - 1, 128], [1, P - j0]])
                nc.sync.dma_start(wt[b][:, j0:P], skA)
                if j0 > 0:
                    # piece B: o in [0, j0): ext[j0+dj, PAD + (o + 2304 - j0) - dj]
                    skB = AP(ext_d, j0 * EXTW + PAD + (P - j0),
                             [[EXTW - 1, 128], [1, j0]])
                    nc.sync.dma_start(wt[b][:, 0:j0], skB)

            # ---- Dykstra (the serial critical path) ----
            al_t = small.tile([DYK_P, DYK_F], F32, tag="al")
            nc.gpsimd.dma_start(al_t[:], al_d.ap())
            zeros_t = small.tile([DYK_P, DYK_F], F32, tag="zeros")
            nc.vector.memset(zeros_t[:], 0.0)
            ones_q = small.tile([DYK_P, DYK_P], F32, tag="onesq")
            nc.vector.memset(ones_q[:], 1.0 / float(P))
            w = small.tile([DYK_P, DYK_F], F32, tag="w")
            ctmp = small.tile([DYK_P, DYK_F], F32, tag="ctmp")
            red = small.tile([DYK_P, 1], F32, tag="red")
            m16 = small.tile([DYK_P, DYK_F], F16, tag="m16")

            # w = 100*alpha ; red = per-partition partials of sum(w)
            nc.vector.tensor_scalar(w[:], al_t[:], INV_LR, 0.0,
                                    op0=mybir.AluOpType.mult,
                                    op1=mybir.AluOpType.add,
                                    accum_out=red[:])
            for i in range(NUM_ITER):
                # s = sum(red) / n, broadcast to DYK_P partitions
                s_ps = dkp.tile([DYK_P, 1], F32, tag="dk")
                nc.tensor.matmul(s_ps[:], ones_q[:], red[:],
                                 start=True, stop=True)
                # w += k/n - s
                nc.vector.tensor_scalar(w[:], w[:], s_ps[:], K_OVER_N,
                                        op0=mybir.AluOpType.subtract,
                                        op1=mybir.AluOpType.add)
                if i < NUM_ITER - 1:
                    # ctmp = clip(w, 0, 1); red = partials of sum(ctmp)
                    nc.vector.scalar_tensor_tensor(
                        ctmp[:], w[:], 1.0, zeros_t[:],
                        op0=mybir.AluOpType.min,
                        op1=mybir.AluOpType.max,
                        accum_out=red[:])
            # m (fp16) = clip(w_50, 0, 1)
            nc.vector.scalar_tensor_tensor(m16[:], w[:], 1.0, zeros_t[:],
                                           op0=mybir.AluOpType.min,
                                           op1=mybir.AluOpType.max)

            # ---- m -> m_rep (130x replicated in DRAM for the skewed read)
            mw0 = nc.gpsimd.dma_start(
                AP(mrep_d, 0, [[DYK_F, DYK_P], [1, DYK_F]]), m16[:])
            rep_engines = [nc.sync, nc.scalar, nc.gpsimd]
            rep_writes = [mw0]
            start = 1
            for ei, eng in enumerate(rep_engines):
                ncopy = 43
                mw = eng.dma_start(
                    AP(mrep_d, P * start, [[P, ncopy], [1, P]]),
                    AP(mrep_d, 0, [[0, ncopy], [1, P]]))
                tile.add_dep_helper(mw.ins, mw0.ins, reason="m_rep RAW")
                rep_writes.append(mw)
                start += ncopy
            assert start == 130

            # ---- skewed m broadcast + mask apply ----
            # msk[b][dj, o] = m_rep[(P - j0) + dj*2303 + o] = m[(o - dj - j0) % P]
            msk_engines = [nc.gpsimd, nc.sync, nc.scalar]
            for b in range(NJ):
                j0 = 128 * b
                msk = mskp.tile([128, P], F16, tag="mskp", name=f"msk{b}")
                mr = msk_engines[b % 3].dma_start(
                    msk[:], AP(mrep_d, P - j0, [[P - 1, 128], [1, P]]))
                for mw in rep_writes:
                    tile.add_dep_helper(mr.ins, mw.ins, reason="m_rep RAW")
                nc.vector.tensor_tensor(wt[b][:], wt[b][:], msk[:],
                                        op=mybir.AluOpType.mult)

            # ---- main matmul: out[t, o] = sum_j x[t, j] * W.T[j, o] ----
            for tt in range(NT):
                row = orow.tile([128, O], F16, tag="orow")
                for ci, (o0, o1) in enumerate(O_CHUNKS):
                    ps = ps8.tile([128, 512], F32, tag="ps8")
                    cw = o1 - o0
                    for b in range(NJ):
                        nc.tensor.matmul(
                            ps[:, 0:cw],
                            xt[b][:, 128 * tt:128 * (tt + 1)],
                            wt[b][:, o0:o1],
                            start=(b == 0), stop=(b == NJ - 1),
                        )
                    if ci % 2 == 0:
                        nc.vector.tensor_copy(row[:, o0:o1], ps[:, 0:cw])
                    else:
                        nc.scalar.copy(row[:, o0:o1], ps[:, 0:cw])
                nc.scalar.dma_start(out_r[tt], row[:])

    nc.compile()
    return nc


_CACHE = {}


def _get_program():
    if "nc" not in _CACHE:
        _CACHE["nc"] = build_program()
    return _CACHE["nc"]


def make_in_maps(x, V, alpha):
    """Host-side layout marshaling: transpose/cast only, no arithmetic."""
    xf = np.asarray(x, dtype=np.float32).reshape(T_FULL, D)
    VT = np.asarray(V, dtype=np.float32).T            # [768, 2304]
    ext = np.empty((D, EXTW), np.float16)
    ext[:, PAD:] = VT
    ext[:, :PAD] = VT[:, P - PAD:]
    al = np.ascontiguousarray(
        np.asarray(alpha, dtype=np.float32).reshape(DYK_P, DYK_F))
    in_maps = []
    for c in range(N_CORES):
        xt = xf[T * c:T * (c + 1)].T.astype(np.float16)  # [768, 2048] C-contig
        in_maps.append({"xt": xt, "ext": ext, "al": al})
    return in_maps


def gather_out(res):
    out = np.concatenate(
        [res.results[c]["out"].astype(np.float32) for c in range(N_CORES)],
        axis=0)
    return out.reshape(16, 1024, O)


def kernel(x, V, alpha):
    nc = _get_program()
    in_maps = make_in_maps(x, V, alpha)
    res = bass_utils.run_bass_kernel_spmd(nc, in_maps,
                                          core_ids=list(range(N_CORES)))
    return gather_out(res)
